# revision 11
# baseline (speedup 1.0000x reference)
"""Cross-attention kernel for Trainium2 (8 NeuronCores, SPMD data-parallel).

Problem: B=4, C=128, 64x64 spatial (N=4096 tokens), 4 heads of dim 32.
  q = Wq @ query; k = Wk @ key; v = Wv @ key   (1x1 convs == channel matmuls)
  out = softmax(q^T k / sqrt(32)) @ v          (per batch*head)

Sharding: 16 (batch, head) jobs -> 2 per core. Core i handles batch i//2,
heads {2*(i%2), 2*(i%2)+1} i.e. output channels [64*(i%2), 64*(i%2)+64).

Structure (per core):
  - The k-projection is folded into the q side on the host:
    scoresT = kin^T (M_h qin) with M_h = log2(e)/sqrt(32) * Wk_h^T Wq_h,
    so raw kin serves as the QK lhsT (no k-projection on device) and only
    t_h = M_h qin (the "q" side, full 128 rows) is projected and cast.
  - QK: scoresT[nk_chunk=128, nq_block=512] = kin_chunk(lhsT) @ t_block,
    K=128 contraction, bf16, scores arrive in the log2 domain.
  - exp: PSUM->SBUF drain split between DVE (one-op Schraudolph exp2:
    int16 <- y*128 + bias, bits reinterpreted as bf16) and ACT (exact
    table exp with scale=ln2), weighted by modeled engine rates.
  - PV flipped: ctx[nq=128, 33] += probsT_chunk(lhsT, stationary) @ v_aug
    (moving, 32 v columns + 1 ones column for the softmax denominator), so
    each chunk's matmul streams only 33 columns instead of 512. v itself is
    projected flipped (v^T[nk, d] = kin_chunk^T(lhsT) @ Wv_h, 32 cols).
  - host: normalize by the denominator column and transpose to [C, N].
"""

import functools
import math

import numpy as np

NCORES = 8
B, C, HS, WS = 4, 128, 64, 64
N = HS * WS  # 4096 tokens
NUM_HEADS = 4
DH = 32  # head dim
HPC = 2  # heads per core

NQB = 512  # nq per QK matmul (one PSUM bank of f32)
NKC = 128  # nk chunk (PV contraction tile)
N_BLOCKS = N // NQB  # 8
N_CHUNKS = N // NKC  # 32
VTW = 33  # v^T tile width: 32 v cols + 1 ones col (denominator)

# Schraudolph exp2 in bf16: i16 = cvt(y*128 + (16256 - C)); bits = bf16 ~ 2^y
EXP2_A = 128.0
EXP2_B = 16256.0 - 5.25

# exp work split across ACT/DVE proportional to modeled per-tile rates
# (GPSIMD cannot access PSUM per the BIR verifier)
EXP_W = {"A": 1.0 / 1038.0, "D": 1.0 / 1192.0}


def _exp_schedule(n):
    """Weighted round-robin assignment of n exp tiles to engines."""
    acc = {k: 0.0 for k in EXP_W}
    tot = sum(EXP_W.values())
    out = []
    for _ in range(n):
        for k in EXP_W:
            acc[k] += EXP_W[k] / tot
        pick = max(acc, key=lambda k: acc[k])
        acc[pick] -= 1.0
        out.append(pick)
    return out


def _f32(x):
    return np.ascontiguousarray(np.asarray(x, dtype=np.float32))


def _bf16(x):
    import ml_dtypes

    return np.ascontiguousarray(
        np.asarray(x, dtype=np.float32).astype(ml_dtypes.bfloat16)
    )


@functools.lru_cache(maxsize=1)
def _build_program():
    from contextlib import ExitStack

    import concourse.tile as tile
    from concourse import bacc, mybir
    from concourse.bass import ts

    f32 = mybir.dt.float32
    bf16 = mybir.dt.bfloat16
    i16 = mybir.dt.int16
    AF = mybir.ActivationFunctionType
    ALU = mybir.AluOpType

    nc = bacc.Bacc(
        "TRN2",
        target_bir_lowering=False,
        debug=False,
        enable_asserts=False,
        num_devices=NCORES,
    )

    qin = nc.dram_tensor("qin", [128, N], bf16, kind="ExternalInput").ap()
    kin = nc.dram_tensor("kin", [128, N], bf16, kind="ExternalInput").ap()
    # win: [m_h0 (128) | m_h1 (128) | wv_h0 (32) | wv_h1 (32)]
    # m_h = log2(e)/sqrt(DH) * Wq_h^T @ Wk_h  (used as lhsT for the t-proj)
    win = nc.dram_tensor("win", [128, 320], bf16, kind="ExternalInput").ap()

    # per (h, nq-block): ctx rows [nq=128 x 4 j-tiles], cols 32 ctx + 1 den
    out_ctx = nc.dram_tensor(
        "out_ctx", [HPC * N_BLOCKS, 128, 4 * VTW], f32, kind="ExternalOutput"
    ).ap()

    ln2 = math.log(2.0)

    with tile.TileContext(nc) as tc, ExitStack() as ctx:
        persist = ctx.enter_context(tc.tile_pool(name="persist", bufs=1))

        win_sb = persist.tile([128, 320], bf16)
        nc.sync.dma_start(out=win_sb, in_=win)

        qin_sb = persist.tile([128, N], bf16)
        kin_sb = persist.tile([128, N], bf16)
        for t in range(4):
            nc.sync.dma_start(
                out=kin_sb[:, ts(t, N // 4)], in_=kin[:, ts(t, N // 4)]
            )
            nc.sync.dma_start(
                out=qin_sb[:, ts(t, N // 4)], in_=qin[:, ts(t, N // 4)]
            )

        # persistent projections: t_h = M_h qin (q side), v^T per head
        tz = [persist.tile([128, N], bf16, name=f"tz{h}") for h in range(HPC)]
        vt = [
            persist.tile([128, VTW * N_CHUNKS], bf16, name=f"vt{h}")
            for h in range(HPC)
        ]
        for h in range(HPC):
            nc.gpsimd.memset(vt[h], 1.0)

        sc_pool = ctx.enter_context(tc.tile_pool(name="sc", bufs=3, space="PSUM"))
        ctx_pool = ctx.enter_context(tc.tile_pool(name="ctxp", bufs=2, space="PSUM"))
        ex_pool = ctx.enter_context(tc.tile_pool(name="ex", bufs=10))
        ob_pool = ctx.enter_context(tc.tile_pool(name="obp", bufs=4))

        def cp(i, dst, src):
            if i % 2 == 0:
                nc.vector.tensor_copy(dst, src)
            else:
                nc.scalar.copy(dst, src)

        # ---- projections ----
        # v flipped: out [nk=128, d=32] per chunk; grouped by input-DMA chunk
        # (8 nk chunks per group) so PV can start before all of kin lands.
        ncp = 0
        for g in range(4):
            pv = sc_pool.tile([128, 2 * NQB], f32, name="pv", tag="sc")
            for h in range(HPC):
                for c8 in range(8):
                    c = 8 * g + c8
                    nc.tensor.matmul(
                        out=pv[:, NQB * h + 32 * c8 : NQB * h + 32 * (c8 + 1)],
                        lhsT=kin_sb[:, ts(c, NKC)],
                        rhs=win_sb[:, 256 + 32 * h : 256 + 32 * (h + 1)],
                        start=True,
                        stop=True,
                    )
            for h in range(HPC):
                src3 = pv[:, NQB * h : NQB * h + 256].rearrange(
                    "p (c w) -> p c w", c=8
                )
                dst = vt[h][:, VTW * 8 * g : VTW * 8 * (g + 1)]
                dst3 = dst.rearrange("p (c w) -> p c w", c=8)
                cp(ncp, dst3[:, :, 0:32], src3)
                ncp += 1
        for h in range(HPC):
            for t in range(4):
                pq = sc_pool.tile([128, 2 * NQB], f32, name="pq", tag="sc")
                for u in range(2):
                    nc.tensor.matmul(
                        out=pq[:, ts(u, NQB)],
                        lhsT=win_sb[:, 128 * h : 128 * (h + 1)],
                        rhs=qin_sb[:, 2 * NQB * t + NQB * u : 2 * NQB * t + NQB * (u + 1)],
                        start=True,
                        stop=True,
                    )
                cp(ncp, tz[h][:, ts(t, 2 * NQB)], pq)
                ncp += 1

        # ---- attention ----
        exp_sched = _exp_schedule(HPC * N_BLOCKS * (N_CHUNKS // 2))
        nexp = 0
        for h in range(HPC):
            for b in range(N_BLOCKS):
                ctx_ps = ctx_pool.tile([128, 4 * VTW], f32, name="ctx_ps")
                for cc in range(N_CHUNKS // 2):
                    sc = sc_pool.tile([128, 2 * NQB], f32, name="sc", tag="sc")
                    for u in range(2):
                        c = 2 * cc + u
                        nc.tensor.matmul(
                            out=sc[:, ts(u, NQB)],
                            lhsT=kin_sb[:, ts(c, NKC)],
                            rhs=tz[h][:, ts(b, NQB)],
                            start=True,
                            stop=True,
                        )
                    ex = ex_pool.tile([128, 2 * NQB], bf16, name="ex")
                    eng = exp_sched[nexp]
                    nexp += 1
                    if eng == "A":
                        nc.scalar.activation(ex, sc, AF.Exp, scale=ln2)
                    else:
                        nc.vector.tensor_scalar(
                            ex.bitcast(i16), sc, EXP2_A, EXP2_B,
                            op0=ALU.mult, op1=ALU.add,
                        )
                    for u in range(2):
                        c = 2 * cc + u
                        for j in range(4):
                            # NOTE: start=True clears has_written BANK-wide,
                            # so only the tile's very first matmul may set it
                            # (the bit-clear makes every region's first write
                            # an overwrite, later writes accumulate).
                            nc.tensor.matmul(
                                out=ctx_ps[:, ts(j, VTW)],
                                lhsT=ex[:, NQB * u + NKC * j : NQB * u + NKC * (j + 1)],
                                rhs=vt[h][:, ts(c, VTW)],
                                start=(c == 0 and j == 0),
                                stop=(c == N_CHUNKS - 1 and j == 3),
                                skip_group_check=True,
                            )
                ob = ob_pool.tile([128, 4 * VTW], f32, name="ob")
                cp(ncp, ob, ctx_ps)
                ncp += 1
                nc.sync.dma_start(out=out_ctx[h * N_BLOCKS + b], in_=ob)

    nc.compile()
    return nc


def _shard_inputs(query, key, Wq, Wk, Wv):
    query = _f32(query).reshape(B, C, N)
    key = _f32(key).reshape(B, C, N)
    Wq, Wk, Wv = _f32(Wq), _f32(Wk), _f32(Wv)

    scale = math.log2(math.e) / math.sqrt(DH)
    in_maps = []
    for core in range(NCORES):
        b, half = core // 2, core % 2
        win = np.zeros((128, 320), np.float32)
        for hl in range(HPC):
            ch0 = 64 * half + 32 * hl
            wq_h = Wq[ch0 : ch0 + 32, :]  # [32, 128]
            wk_h = Wk[ch0 : ch0 + 32, :]
            win[:, 128 * hl : 128 * (hl + 1)] = scale * (wq_h.T @ wk_h)
            win[:, 256 + 32 * hl : 256 + 32 * (hl + 1)] = Wv[ch0 : ch0 + 32, :].T
        in_maps.append(
            {
                "qin": _bf16(query[b]),
                "kin": _bf16(key[b]),
                "win": _bf16(win),
            }
        )
    return in_maps


def _run(in_maps, trace=False):
    from concourse import bass_utils

    nc = _build_program()
    return bass_utils.run_bass_kernel_spmd(
        nc, in_maps, core_ids=list(range(NCORES)), trace=trace
    )


def _assemble(results):
    out = np.empty((B, C, N), np.float32)
    for core in range(NCORES):
        b, half = core // 2, core % 2
        r = results[core]
        t = np.asarray(r["out_ctx"], np.float32)  # [16, 128, 132]
        t = t.reshape(HPC, N_BLOCKS, 128, 4, VTW)
        ctx = t[..., :32]  # [h, b8, p, j, d]
        den = t[..., 32]  # [h, b8, p, j]
        # nq index = b8*512 + j*128 + p -> order (b8, j, p)
        ctx = np.transpose(ctx, (0, 1, 3, 2, 4)).reshape(HPC, N, 32)
        den = np.transpose(den, (0, 1, 3, 2)).reshape(HPC, N)
        for hl in range(HPC):
            ch0 = 64 * half + 32 * hl
            out[b, ch0 : ch0 + 32, :] = (ctx[hl] / den[hl][:, None]).T
    return out.reshape(B, C, HS, WS)


def kernel(query, key, Wq, Wk, Wv):
    in_maps = _shard_inputs(query, key, Wq, Wk, Wv)
    res = _run(in_maps)
    return _assemble(res.results)


# revision 12
# speedup vs baseline: 1.0552x; 1.0552x over previous
"""Cross-attention kernel for Trainium2 (8 NeuronCores, SPMD data-parallel).

Problem: B=4, C=128, 64x64 spatial (N=4096 tokens), 4 heads of dim 32.
  q = Wq @ query; k = Wk @ key; v = Wv @ key   (1x1 convs == channel matmuls)
  out = softmax(q^T k / sqrt(32)) @ v          (per batch*head)

Sharding: 16 (batch, head) jobs -> 2 per core. Core i handles batch i//2,
heads {2*(i%2), 2*(i%2)+1} i.e. output channels [64*(i%2), 64*(i%2)+64).

Structure (per core):
  - The k-projection is folded into the q side on the host:
    scoresT = kin^T (M_h qin) with M_h = log2(e)/sqrt(32) * Wk_h^T Wq_h,
    so raw kin serves as the QK lhsT (no k-projection on device) and only
    t_h = M_h qin (the "q" side, full 128 rows) is projected and cast.
  - QK: scoresT[nk_chunk=128, nq_block=512] = kin_chunk(lhsT) @ t_block,
    K=128 contraction, bf16, scores arrive in the log2 domain.
  - exp: PSUM->SBUF drain split between DVE (one-op Schraudolph exp2:
    int16 <- y*128 + bias, bits reinterpreted as bf16) and ACT (exact
    table exp with scale=ln2), weighted by modeled engine rates.
  - PV flipped: ctx[nq=128, 33] += probsT_chunk(lhsT, stationary) @ v_aug
    (moving, 32 v columns + 1 ones column for the softmax denominator), so
    each chunk's matmul streams only 33 columns instead of 512. v itself is
    projected flipped (v^T[nk, d] = kin_chunk^T(lhsT) @ Wv_h, 32 cols).
  - host: normalize by the denominator column and transpose to [C, N].
"""

import functools
import math

import numpy as np

NCORES = 8
B, C, HS, WS = 4, 128, 64, 64
N = HS * WS  # 4096 tokens
NUM_HEADS = 4
DH = 32  # head dim
HPC = 2  # heads per core

NQB = 512  # nq per QK matmul (one PSUM bank of f32)
NKC = 128  # nk chunk (PV contraction tile)
N_BLOCKS = N // NQB  # 8
N_CHUNKS = N // NKC  # 32
VTW = 33  # v^T tile width: 32 v cols + 1 ones col (denominator)

# Schraudolph exp2 in bf16: i16 = cvt(y*128 + (16256 - C)); bits = bf16 ~ 2^y
EXP2_A = 128.0
EXP2_B = 16256.0 - 5.25

# exp tiles strictly alternate ACT/DVE: PV consumes exp results in order,
# so adjacent same-engine tiles serialize the pipeline; strict alternation
# lets consecutive pairs complete concurrently on the two engines.
def _exp_schedule(n):
    return ["A" if i % 2 == 0 else "D" for i in range(n)]


def _f32(x):
    return np.ascontiguousarray(np.asarray(x, dtype=np.float32))


def _bf16(x):
    import ml_dtypes

    return np.ascontiguousarray(
        np.asarray(x, dtype=np.float32).astype(ml_dtypes.bfloat16)
    )


@functools.lru_cache(maxsize=1)
def _build_program():
    from contextlib import ExitStack

    import concourse.tile as tile
    from concourse import bacc, mybir
    from concourse.bass import ts

    f32 = mybir.dt.float32
    bf16 = mybir.dt.bfloat16
    i16 = mybir.dt.int16
    AF = mybir.ActivationFunctionType
    ALU = mybir.AluOpType

    nc = bacc.Bacc(
        "TRN2",
        target_bir_lowering=False,
        debug=False,
        enable_asserts=False,
        num_devices=NCORES,
    )

    qin = nc.dram_tensor("qin", [128, N], bf16, kind="ExternalInput").ap()
    kin = nc.dram_tensor("kin", [128, N], bf16, kind="ExternalInput").ap()
    # win: [m_h0 (128) | m_h1 (128) | wv_h0 (32) | wv_h1 (32)]
    # m_h = log2(e)/sqrt(DH) * Wq_h^T @ Wk_h  (used as lhsT for the t-proj)
    win = nc.dram_tensor("win", [128, 320], bf16, kind="ExternalInput").ap()

    # per (h, nq-block): ctx rows [nq=128 x 4 j-tiles], cols 32 ctx + 1 den
    out_ctx = nc.dram_tensor(
        "out_ctx", [HPC * N_BLOCKS, 128, 4 * VTW], f32, kind="ExternalOutput"
    ).ap()

    ln2 = math.log(2.0)

    with tile.TileContext(nc) as tc, ExitStack() as ctx:
        persist = ctx.enter_context(tc.tile_pool(name="persist", bufs=1))

        win_sb = persist.tile([128, 320], bf16)
        nc.sync.dma_start(out=win_sb, in_=win)

        qin_sb = persist.tile([128, N], bf16)
        kin_sb = persist.tile([128, N], bf16)
        for t in range(4):
            nc.sync.dma_start(
                out=kin_sb[:, ts(t, N // 4)], in_=kin[:, ts(t, N // 4)]
            )
            nc.sync.dma_start(
                out=qin_sb[:, ts(t, N // 4)], in_=qin[:, ts(t, N // 4)]
            )

        # persistent projections: t_h = M_h qin (q side), v^T per head
        tz = [persist.tile([128, N], bf16, name=f"tz{h}") for h in range(HPC)]
        vt = [
            persist.tile([128, VTW * N_CHUNKS], bf16, name=f"vt{h}")
            for h in range(HPC)
        ]
        for h in range(HPC):
            nc.gpsimd.memset(vt[h], 1.0)

        sc_pool = ctx.enter_context(tc.tile_pool(name="sc", bufs=3, space="PSUM"))
        ctx_pool = ctx.enter_context(tc.tile_pool(name="ctxp", bufs=2, space="PSUM"))
        ex_pool = ctx.enter_context(tc.tile_pool(name="ex", bufs=10))
        ob_pool = ctx.enter_context(tc.tile_pool(name="obp", bufs=4))

        def cp(i, dst, src):
            if i % 2 == 0:
                nc.vector.tensor_copy(dst, src)
            else:
                nc.scalar.copy(dst, src)

        # ---- projections ----
        # v flipped: out [nk=128, d=32] per chunk; grouped by input-DMA chunk
        # (8 nk chunks per group) so PV can start before all of kin lands.
        ncp = 0
        for g in range(4):
            pv = sc_pool.tile([128, 2 * NQB], f32, name="pv", tag="sc")
            for h in range(HPC):
                for c8 in range(8):
                    c = 8 * g + c8
                    nc.tensor.matmul(
                        out=pv[:, NQB * h + 32 * c8 : NQB * h + 32 * (c8 + 1)],
                        lhsT=kin_sb[:, ts(c, NKC)],
                        rhs=win_sb[:, 256 + 32 * h : 256 + 32 * (h + 1)],
                        start=True,
                        stop=True,
                    )
            for h in range(HPC):
                src3 = pv[:, NQB * h : NQB * h + 256].rearrange(
                    "p (c w) -> p c w", c=8
                )
                dst = vt[h][:, VTW * 8 * g : VTW * 8 * (g + 1)]
                dst3 = dst.rearrange("p (c w) -> p c w", c=8)
                cp(ncp, dst3[:, :, 0:32], src3)
                ncp += 1
        for h in range(HPC):
            for t in range(4):
                pq = sc_pool.tile([128, 2 * NQB], f32, name="pq", tag="sc")
                for u in range(2):
                    nc.tensor.matmul(
                        out=pq[:, ts(u, NQB)],
                        lhsT=win_sb[:, 128 * h : 128 * (h + 1)],
                        rhs=qin_sb[:, 2 * NQB * t + NQB * u : 2 * NQB * t + NQB * (u + 1)],
                        start=True,
                        stop=True,
                    )
                cp(ncp, tz[h][:, ts(t, 2 * NQB)], pq)
                ncp += 1

        # ---- attention ----
        exp_sched = _exp_schedule(HPC * N_BLOCKS * (N_CHUNKS // 2))
        nexp = 0
        for h in range(HPC):
            for b in range(N_BLOCKS):
                ctx_ps = ctx_pool.tile([128, 4 * VTW], f32, name="ctx_ps")
                for cc in range(N_CHUNKS // 2):
                    sc = sc_pool.tile([128, 2 * NQB], f32, name="sc", tag="sc")
                    for u in range(2):
                        c = 2 * cc + u
                        nc.tensor.matmul(
                            out=sc[:, ts(u, NQB)],
                            lhsT=kin_sb[:, ts(c, NKC)],
                            rhs=tz[h][:, ts(b, NQB)],
                            start=True,
                            stop=True,
                        )
                    ex = ex_pool.tile([128, 2 * NQB], bf16, name="ex")
                    eng = exp_sched[nexp]
                    nexp += 1
                    if eng == "A":
                        nc.scalar.activation(ex, sc, AF.Exp, scale=ln2)
                    else:
                        nc.vector.tensor_scalar(
                            ex.bitcast(i16), sc, EXP2_A, EXP2_B,
                            op0=ALU.mult, op1=ALU.add,
                        )
                    for u in range(2):
                        c = 2 * cc + u
                        for j in range(4):
                            # NOTE: start=True clears has_written BANK-wide,
                            # so only the tile's very first matmul may set it
                            # (the bit-clear makes every region's first write
                            # an overwrite, later writes accumulate).
                            nc.tensor.matmul(
                                out=ctx_ps[:, ts(j, VTW)],
                                lhsT=ex[:, NQB * u + NKC * j : NQB * u + NKC * (j + 1)],
                                rhs=vt[h][:, ts(c, VTW)],
                                start=(c == 0 and j == 0),
                                stop=(c == N_CHUNKS - 1 and j == 3),
                                skip_group_check=True,
                            )
                ob = ob_pool.tile([128, 4 * VTW], f32, name="ob")
                nc.scalar.copy(ob, ctx_ps)  # fits ACT's per-block slack
                nc.sync.dma_start(out=out_ctx[h * N_BLOCKS + b], in_=ob)

    nc.compile()
    return nc


def _shard_inputs(query, key, Wq, Wk, Wv):
    query = _f32(query).reshape(B, C, N)
    key = _f32(key).reshape(B, C, N)
    Wq, Wk, Wv = _f32(Wq), _f32(Wk), _f32(Wv)

    scale = math.log2(math.e) / math.sqrt(DH)
    in_maps = []
    for core in range(NCORES):
        b, half = core // 2, core % 2
        win = np.zeros((128, 320), np.float32)
        for hl in range(HPC):
            ch0 = 64 * half + 32 * hl
            wq_h = Wq[ch0 : ch0 + 32, :]  # [32, 128]
            wk_h = Wk[ch0 : ch0 + 32, :]
            win[:, 128 * hl : 128 * (hl + 1)] = scale * (wq_h.T @ wk_h)
            win[:, 256 + 32 * hl : 256 + 32 * (hl + 1)] = Wv[ch0 : ch0 + 32, :].T
        in_maps.append(
            {
                "qin": _bf16(query[b]),
                "kin": _bf16(key[b]),
                "win": _bf16(win),
            }
        )
    return in_maps


def _run(in_maps, trace=False):
    from concourse import bass_utils

    nc = _build_program()
    return bass_utils.run_bass_kernel_spmd(
        nc, in_maps, core_ids=list(range(NCORES)), trace=trace
    )


def _assemble(results):
    out = np.empty((B, C, N), np.float32)
    for core in range(NCORES):
        b, half = core // 2, core % 2
        r = results[core]
        t = np.asarray(r["out_ctx"], np.float32)  # [16, 128, 132]
        t = t.reshape(HPC, N_BLOCKS, 128, 4, VTW)
        ctx = t[..., :32]  # [h, b8, p, j, d]
        den = t[..., 32]  # [h, b8, p, j]
        # nq index = b8*512 + j*128 + p -> order (b8, j, p)
        ctx = np.transpose(ctx, (0, 1, 3, 2, 4)).reshape(HPC, N, 32)
        den = np.transpose(den, (0, 1, 3, 2)).reshape(HPC, N)
        for hl in range(HPC):
            ch0 = 64 * half + 32 * hl
            out[b, ch0 : ch0 + 32, :] = (ctx[hl] / den[hl][:, None]).T
    return out.reshape(B, C, HS, WS)


def kernel(query, key, Wq, Wk, Wv):
    in_maps = _shard_inputs(query, key, Wq, Wk, Wv)
    res = _run(in_maps)
    return _assemble(res.results)
